# revision 1
# baseline (speedup 1.0000x reference)
"""BertAttention (cross-attention, eval) on 8 Trainium2 NeuronCores.

Problem: B=4, SQ=SK=2048, HID=1024, NH=16, HD=64.
  q = hidden @ Wq + bq ; k = ctx @ Wk + bk ; v = ctx @ Wv + bv
  out = softmax(q k^T / 8) v        (per head), heads re-merged.

Sharding (no collectives needed): 8 cores = 4 batches x 2 head-groups.
Core c handles batch b = c//2 and heads hs..hs+8 where hs = (c%2)*8.

Math rearrangement used by the kernel (all exact):
  * softmax is shift-invariant per row, so the k-bias terms q@bk^T and
    bq@bk^T cancel.  Only the rank-1 term rT[k] = bq . K[k,:]/8 survives;
    rT = C @ (Wk @ bq)/8 (+ const that also cancels) and is produced on
    device as extra columns of the V projection, then fed to exp() as a
    per-partition bias.
  * exp() is applied without max subtraction (scores ~ N(0,1), safe in f32).
  * P@V is computed unnormalized with a ones-column appended to V, so the
    PSUM accumulator row 64 holds the softmax denominator; a reciprocal +
    broadcast multiply normalizes at the end, then + bv.
Layouts: scores are built transposed (k on partitions, q free) so exp()
output PT feeds the P@V matmul directly as the moving operand - no
transposes anywhere on device.  The host hands the kernel pre-transposed
x^T / c^T in bf16 and re-transposes the [512, 2048] per-core output.
"""

import numpy as np
import ml_dtypes

import concourse.bass as bass
import concourse.mybir as mybir
import concourse.tile as tile
from concourse import bacc
from concourse.bass_utils import run_bass_kernel_spmd

P = 128
B, SQ, SK, HID, NH = 4, 2048, 2048, 1024, 16
HD = 64
N_CORES = 8
NHC = NH // 2          # heads per core = 8
DW = NHC * HD          # per-core output width = 512
VW = NHC * (HD + 1)    # V block width per k-chunk (64 vals + 1 ones col per head)

_BF = ml_dtypes.bfloat16


def build_nc(sq=SQ, sk=SK, hid=HID, nhc=NHC, reps=1, interleave_vproj=True):
    """Build the single-core Bass program (same program runs SPMD on all 8).

    reps > 1 repeats the whole computation (including DMAs) in one NEFF;
    used only for differential wall-clock timing of the kernel body.
    """
    hd = HD
    cc_n = hid // P          # contraction chunks (8)
    kc_n = sk // P           # key chunks (16)
    pairs = nhc // 2
    dw = nhc * hd
    vw = nhc * (hd + 1)
    q4_n = sq // 512         # 512-wide q tiles for projections
    q2_n = sq // 1024        # 1024-wide q tiles for attention

    bf = mybir.dt.bfloat16
    f32 = mybir.dt.float32
    Exp = mybir.ActivationFunctionType.Exp
    MULT = mybir.AluOpType.mult

    nc = bacc.Bacc("TRN2", target_bir_lowering=False, debug=False)

    xT = nc.dram_tensor("xT", [hid, sq], bf, kind="ExternalInput").ap()
    cT = nc.dram_tensor("cT", [hid, sk], bf, kind="ExternalInput").ap()
    wq = nc.dram_tensor("wq", [hid, dw], bf, kind="ExternalInput").ap()
    wk = nc.dram_tensor("wk", [hid, dw], bf, kind="ExternalInput").ap()
    # wv: [hid, dw + nhc]; last nhc columns produce rT (exp bias) per head
    wv = nc.dram_tensor("wv", [hid, dw + nhc], bf, kind="ExternalInput").ap()
    bv = nc.dram_tensor("bv", [dw], f32, kind="ExternalInput").ap()
    out = nc.dram_tensor("out", [dw, sq], f32, kind="ExternalOutput").ap()

    with tile.TileContext(nc) as tc:
        with (
            tc.tile_pool(name="const", bufs=1) as cpool,
            tc.tile_pool(name="qk", bufs=2) as qkpool,
            tc.tile_pool(name="pt", bufs=6) as ptpool,
            tc.tile_pool(name="work", bufs=3) as wpool,
            tc.tile_pool(name="psum", bufs=2, space="PSUM") as pspool,
        ):
            for _rep in range(reps):
                xT_sb = cpool.tile([P, cc_n * sq], bf, name="xT_sb")
                cT_sb = cpool.tile([P, cc_n * sk], bf, name="cT_sb")
                wq_sb = cpool.tile([P, cc_n * dw], bf, name="wq_sb")
                wk_sb = cpool.tile([P, cc_n * dw], bf, name="wk_sb")
                wv_sb = cpool.tile([P, cc_n * (dw + nhc)], bf, name="wv_sb")
                v_sb = cpool.tile([P, kc_n * vw], bf, name="v_sb")
                rt_sb = cpool.tile([P, kc_n * nhc], f32, name="rt_sb")
                bv_sb = cpool.tile([hd, nhc], f32, name="bv_sb")

                # DMA issue order = first-consumer order: the V projection
                # (first PE work) needs cT+wv; Q-proj needs wq+xT; K-proj wk.
                for cc in range(cc_n):
                    nc.sync.dma_start(
                        wv_sb[:, cc * (dw + nhc):(cc + 1) * (dw + nhc)],
                        wv[cc * P:(cc + 1) * P, :])
                    nc.sync.dma_start(cT_sb[:, cc * sk:(cc + 1) * sk],
                                      cT[cc * P:(cc + 1) * P, :])
                for cc in range(cc_n):
                    nc.sync.dma_start(wq_sb[:, cc * dw:(cc + 1) * dw],
                                      wq[cc * P:(cc + 1) * P, :])
                    nc.sync.dma_start(xT_sb[:, cc * sq:(cc + 1) * sq],
                                      xT[cc * P:(cc + 1) * P, :])
                for cc in range(cc_n):
                    nc.sync.dma_start(wk_sb[:, cc * dw:(cc + 1) * dw],
                                      wk[cc * P:(cc + 1) * P, :])
                nc.sync.dma_start(bv_sb[:, :], bv.rearrange("(h d) -> d h", d=hd))

                # every 65th column of v_sb is a ones column (denominator trick):
                # memset everything to 1.0, the V-projection copies overwrite the
                # first 64 columns of each head block.
                nc.vector.memset(v_sb[:, :], 1.0)

                # ---- V projection (all heads at once) + rT columns ----
                def emit_vproj(kc):
                    pv_ps = pspool.tile([P, 1024], f32, tag="st", name="pv_ps")
                    for cc in range(cc_n):
                        lhs = cT_sb[:, cc * sk + kc * P: cc * sk + kc * P + P]
                        nc.tensor.matmul(
                            pv_ps[:, 0:dw], lhsT=lhs,
                            rhs=wv_sb[:, cc * (dw + nhc): cc * (dw + nhc) + dw],
                            start=(cc == 0), stop=(cc == cc_n - 1))
                        # rT columns go at col 512 = bank 1 of the slot, so their
                        # accumulation group never shares a bank with the V group.
                        nc.tensor.matmul(
                            pv_ps[:, 512:512 + nhc], lhsT=lhs,
                            rhs=wv_sb[:, cc * (dw + nhc) + dw: (cc + 1) * (dw + nhc)],
                            start=(cc == 0), stop=(cc == cc_n - 1))
                    vdst = v_sb[:, kc * vw:(kc + 1) * vw].rearrange(
                        "p (h w) -> p h w", h=nhc)[:, :, 0:hd]
                    vsrc = pv_ps[:, 0:dw].rearrange("p (h w) -> p h w", h=nhc)
                    nc.vector.tensor_copy(vdst, vsrc)
                    nc.vector.tensor_copy(rt_sb[:, kc * nhc:(kc + 1) * nhc],
                                           pv_ps[:, 512:512 + nhc])

                if not interleave_vproj:
                    for kc in range(kc_n):
                        emit_vproj(kc)

                # ---- Q/K projections, emitted in groups so pair p+1's
                # projection hides under pair p's (ACT-bound) attention ----
                n_qg = (sq + 511) // 512
                n_kg = (sk + 511) // 512
                qkt_cache = {}

                def get_qkt(pp):
                    if pp not in qkt_cache:
                        qt = qkpool.tile([P, sq], bf, tag="qt", name=f"qt{pp}")
                        kt = qkpool.tile([P, sk], bf, tag="kt", name=f"kt{pp}")
                        qkt_cache[pp] = (qt, kt)
                    return qkt_cache[pp]

                def emit_proj_group(pp, g):
                    qt, kt = get_qkt(pp)
                    if g < n_qg:
                        t0 = g * 512
                        tw = min(512, sq - t0)
                        q_ps = pspool.tile([P, 1024], f32, tag="st", name="q_ps")
                        for cc in range(cc_n):
                            nc.tensor.matmul(
                                q_ps[:, 0:tw],
                                lhsT=wq_sb[:, cc * dw + pp * P: cc * dw + pp * P + P],
                                rhs=xT_sb[:, cc * sq + t0: cc * sq + t0 + tw],
                                start=(cc == 0), stop=(cc == cc_n - 1))
                        nc.vector.tensor_copy(qt[:, t0:t0 + tw], q_ps[:, 0:tw])
                    else:
                        t0 = (g - n_qg) * 512
                        tw = min(512, sk - t0)
                        k_ps = pspool.tile([P, 1024], f32, tag="st", name="k_ps")
                        for cc in range(cc_n):
                            nc.tensor.matmul(
                                k_ps[:, 0:tw],
                                lhsT=wk_sb[:, cc * dw + pp * P: cc * dw + pp * P + P],
                                rhs=cT_sb[:, cc * sk + t0: cc * sk + t0 + tw],
                                start=(cc == 0), stop=(cc == cc_n - 1))
                        nc.vector.tensor_copy(kt[:, t0:t0 + tw], k_ps[:, 0:tw])

                n_groups = n_qg + n_kg

                for p in range(pairs):
                    h0, h1 = 2 * p, 2 * p + 1
                    if p == 0:
                        for g in range(n_groups):
                            emit_proj_group(0, g)
                    qt_sb, kt_sb = get_qkt(p)
                    qkt_cache.pop(p - 1, None)

                    for q2 in range(q2_n):
                        ctx0 = pspool.tile([P, 1024], f32, tag="ctx", name="ctx0")
                        ctx1 = pspool.tile([P, 1024], f32, tag="ctx", name="ctx1")

                        def emit_pv(kc, pt0, pt1):
                            for qh in range(2):
                                cs = slice(qh * 512, (qh + 1) * 512)
                                nc.tensor.matmul(
                                    ctx0[0:hd + 1, cs],
                                    lhsT=v_sb[:, kc * vw + h0 * (hd + 1): kc * vw + (h0 + 1) * (hd + 1)],
                                    rhs=pt0[:, cs],
                                    start=(kc == 0), stop=(kc == kc_n - 1))
                                nc.tensor.matmul(
                                    ctx1[0:hd + 1, cs],
                                    lhsT=v_sb[:, kc * vw + h1 * (hd + 1): kc * vw + (h1 + 1) * (hd + 1)],
                                    rhs=pt1[:, cs],
                                    start=(kc == 0), stop=(kc == kc_n - 1))

                        # software-pipelined by one chunk: PV(kc-1) is emitted
                        # after QK(kc)/exp(kc).  PE executes in program order,
                        # so emitting PV(kc) here directly would head-of-line
                        # block QK(kc+1) behind a matmul that waits on exp(kc),
                        # starving the (bottleneck) activation engine.
                        prev = None
                        for kc in range(kc_n):
                            # V-projection is interleaved just-in-time into the
                            # very first attention pass (chunk kc is produced
                            # right before its scores), hiding proj startup
                            # under the activation-bound steady state.
                            if interleave_vproj and p == 0 and q2 == 0:
                                emit_vproj(kc)
                            if (p + 1 < pairs and q2 == q2_n - 1
                                    and kc % 2 == 0 and kc // 2 < n_groups):
                                emit_proj_group(p + 1, kc // 2)
                            st0 = pspool.tile([P, 1024], f32, tag="st", name="st0")
                            st1 = pspool.tile([P, 1024], f32, tag="st", name="st1")
                            for qh in range(2):
                                qs = q2 * 1024 + qh * 512
                                # two heads (d=64 each) packed into the PE array
                                nc.tensor.matmul(
                                    st0[:, qh * 512:(qh + 1) * 512],
                                    lhsT=kt_sb[0:64, kc * P:(kc + 1) * P],
                                    rhs=qt_sb[0:64, qs:qs + 512],
                                    start=True, stop=True, tile_position=(0, 0))
                                nc.tensor.matmul(
                                    st1[:, qh * 512:(qh + 1) * 512],
                                    lhsT=kt_sb[64:128, kc * P:(kc + 1) * P],
                                    rhs=qt_sb[64:128, qs:qs + 512],
                                    start=True, stop=True, tile_position=(64, 0))
                            pt0 = ptpool.tile([P, 1024], bf, tag="pt", name="pt0")
                            pt1 = ptpool.tile([P, 1024], bf, tag="pt", name="pt1")
                            nc.scalar.activation(pt0, st0, Exp,
                                                 bias=rt_sb[:, kc * nhc + h0: kc * nhc + h0 + 1])
                            nc.scalar.activation(pt1, st1, Exp,
                                                 bias=rt_sb[:, kc * nhc + h1: kc * nhc + h1 + 1])
                            if prev is not None:
                                emit_pv(*prev)
                            prev = (kc, pt0, pt1)
                        emit_pv(*prev)
                        for hh, ctx_ps in ((0, ctx0), (1, ctx1)):
                            h = 2 * p + hh
                            rec = wpool.tile([1, 1024], f32, tag="rec", name="rec")
                            nc.vector.reciprocal(rec, ctx_ps[hd:hd + 1, :])
                            rec_bc = wpool.tile([hd, 1024], f32, tag="recbc",
                                                name="rec_bc")
                            nc.gpsimd.partition_broadcast(rec_bc[:, :], rec[:, :])
                            o_sb = wpool.tile([hd, 1024], f32, tag="osb", name="o_sb")
                            nc.vector.tensor_tensor(
                                o_sb[:, :], ctx_ps[0:hd, :], rec_bc[:, :], MULT)
                            nc.vector.tensor_scalar_add(o_sb[:, :], o_sb[:, :],
                                                        bv_sb[:, h:h + 1])
                            nc.sync.dma_start(
                                out[p * P + hh * hd: p * P + (hh + 1) * hd,
                                    q2 * 1024:(q2 + 1) * 1024],
                                o_sb[:, :])

    nc.compile()
    return nc


_NC_CACHE = {}


def _get_nc():
    if "nc" not in _NC_CACHE:
        _NC_CACHE["nc"] = build_nc()
    return _NC_CACHE["nc"]


def _prep_core_inputs(hidden_states, context, Wq, bq, Wk, bk, Wv, bv):
    """Host-side shard + layout prep. Returns list of 8 in_maps."""
    scale = 1.0 / np.sqrt(HD)
    xT_b = []
    cT_b = []
    for b in range(B):
        xT_b.append(np.ascontiguousarray(hidden_states[b].T).astype(_BF))
        cT_b.append(np.ascontiguousarray(context[b].T).astype(_BF))
    in_maps = []
    for c in range(N_CORES):
        b = c // 2
        hs = (c % 2) * NHC
        cols = slice(hs * HD, (hs + NHC) * HD)
        wq_c = (Wq[:, cols] * scale).astype(_BF)
        wk_c = Wk[:, cols].astype(_BF)
        # rT producer columns: (Wk_h @ bq_h) * scale  for each head h
        wkr = np.empty((HID, NHC), np.float32)
        for h in range(NHC):
            hcols = slice((hs + h) * HD, (hs + h + 1) * HD)
            wkr[:, h] = (Wk[:, hcols] @ bq[hcols]) * scale
        wv_c = np.concatenate(
            [Wv[:, cols].astype(np.float32), wkr], axis=1).astype(_BF)
        in_maps.append({
            "xT": xT_b[b],
            "cT": cT_b[b],
            "wq": np.ascontiguousarray(wq_c),
            "wk": np.ascontiguousarray(wk_c),
            "wv": np.ascontiguousarray(wv_c),
            "bv": np.ascontiguousarray(bv[cols]).astype(np.float32),
        })
    return in_maps


def kernel(hidden_states, context, Wq, bq, Wk, bk, Wv, bv):
    hidden_states = np.asarray(hidden_states, dtype=np.float32)
    context = np.asarray(context, dtype=np.float32)
    Wq = np.asarray(Wq, dtype=np.float32)
    bq = np.asarray(bq, dtype=np.float32)
    Wk = np.asarray(Wk, dtype=np.float32)
    bk = np.asarray(bk, dtype=np.float32)
    Wv = np.asarray(Wv, dtype=np.float32)
    bv = np.asarray(bv, dtype=np.float32)

    nc = _get_nc()
    in_maps = _prep_core_inputs(hidden_states, context, Wq, bq, Wk, bk, Wv, bv)
    res = run_bass_kernel_spmd(nc, in_maps, list(range(N_CORES)))
    full = np.empty((B, SQ, NH * HD), np.float32)
    for c in range(N_CORES):
        b = c // 2
        hs = (c % 2) * NHC
        cols = slice(hs * HD, (hs + NHC) * HD)
        full[b, :, cols] = res.results[c]["out"].T
    return full

